# revision 10
# baseline (speedup 1.0000x reference)
"""Multi-head causal self-attention on 8 Trainium2 NeuronCores.

Problem: B=4, T=2048, D=1024, H=16 heads, Hd=64. fp32.
Sharding: core c handles batch b = c//2 and head-group g = c%2 (8 heads,
512 channels). Each core computes a partial output (its head-group's
contribution to x @ Wo); the host sums head-group pairs and adds bo.

Per-core algorithm (x is pre-transposed on the host, so no on-chip
transposes are needed anywhere):
  x^T  [D=1024, T]   host-side transpose, plain DMA in (split across queues)
  Q^T  [C=512, T]    = matmul(lhsT=Wq chunk, rhs=x^T)   (head h at partitions
  K^T  [C=512, T]      64*(h%2) .. of chunk h//2)
  V'   [T, 8*65]     = matmul(lhsT=x^T chunk, rhs=Wv), per head [V(64) | 1]
  S^T  [k,q]         = matmul(lhsT=K^T block, rhs=Q^T span)  (k on partitions)
  E = exp((S^T+mask)/8)  on ScalarE, PSUM->SBUF
  ctx' [65, q]       = matmul(lhsT=V' block, rhs=E)  accumulated over k blocks
                       row 64 = softmax denominator (ones-column trick)
  ctx^T normalized via reciprocal + DRAM-bounce broadcast + DVE mult
  out  [T, D]        = matmul(lhsT=ctx^T chunk, rhs=Wo chunk), DMA out
Causality: only k-blocks with k0 <= q_span_end are computed; the <=4
diagonal blocks per span get an additive staircase mask.

Schedule: the attention inner loop is ScalarE-bound (exp), so the Q/K
projections are interleaved per head-pair into the attention sweep
(B1(hp) right before att(*, hp)) and the out-projection of span s is
emitted inside the hp=3 sweep — both give the PE dense work while
ScalarE digests exps.  All PSUM flows through three pools (st 2x2
banks, csA/csB 2x1 banks each = 8 banks); B1 and the out-projection
borrow the cs tiles.  ctx matmuls trail the S matmuls by one k-block
so exp has extra slack.
"""

import sys

for _p in ("/opt/trn_rl_repo", "/root/.axon_site/_ro/trn_rl_repo"):
    if _p not in sys.path:
        sys.path.append(_p)

import numpy as np

import concourse.bacc as bacc
import concourse.mybir as mybir
import concourse.tile as tile
from concourse.bass_utils import run_bass_kernel_spmd

FP32 = mybir.dt.float32
BF16 = mybir.dt.bfloat16
P = 128
T = 2048  # sequence length
D = 1024  # model dim
C = 512   # channels per core (8 heads)
H = 8     # heads per core
HD = 64   # head dim
N_CORES = 8
NSPAN = 4          # q spans of 512
SPAN = 512
NKB = 16           # k blocks of 128

_program = None


def _build():
    nc = bacc.Bacc()
    x_d = nc.declare_dram_parameter("x", [D, T], BF16, isOutput=False)  # x^T
    wq_d = nc.declare_dram_parameter("wq", [D, C], BF16, isOutput=False)
    wk_d = nc.declare_dram_parameter("wk", [D, C], BF16, isOutput=False)
    wv_d = nc.declare_dram_parameter("wv", [D, C], BF16, isOutput=False)
    wo_d = nc.declare_dram_parameter("wo", [C, D], BF16, isOutput=False)
    mask_d = nc.declare_dram_parameter("mask", [P, 1024], BF16, isOutput=False)
    out_d = nc.declare_dram_parameter("out", [T, D], FP32, isOutput=True)

    Exp = mybir.ActivationFunctionType.Exp

    def copy_px(idx, dst, src):
        # alternate PSUM->SBUF copies between ScalarE and VectorE
        if idx % 2 == 0:
            nc.scalar.copy(dst, src)
        else:
            nc.vector.tensor_copy(dst, src)

    from contextlib import ExitStack

    with tile.TileContext(nc) as tc, ExitStack() as persist:
        const_pool = persist.enter_context(tc.tile_pool(name="const", bufs=1))
        qkt_pool = persist.enter_context(tc.tile_pool(name="qkt", bufs=1))
        vp_pool = persist.enter_context(tc.tile_pool(name="vp", bufs=1))
        persist_w = persist.enter_context(tc.tile_pool(name="pw", bufs=1))
        ctxT_pool = persist.enter_context(tc.tile_pool(name="ctxT", bufs=1))

        mask_sb = const_pool.tile([P, 1024], BF16, tag="mask")
        nc.sync.dma_start(mask_sb[:], mask_d[:])
        qt = [qkt_pool.tile([P, T], BF16, tag=f"qt{i}", name=f"qt{i}") for i in range(4)]
        kt = [qkt_pool.tile([P, T], BF16, tag=f"kt{i}", name=f"kt{i}") for i in range(4)]
        vp = [vp_pool.tile([P, H * 65], BF16, tag=f"vp{t}", name=f"vp{t}") for t in range(NKB)]
        ctxT = [ctxT_pool.tile([P, T], BF16, tag=f"ct{i}", name=f"ct{i}")
                for i in range(4)]
        wq_sb = persist_w.tile([P, 8, C], BF16, tag="wq")
        wk_sb = persist_w.tile([P, 8, C], BF16, tag="wk")
        wv_sb = persist_w.tile([P, 8, C], BF16, tag="wv")
        wo_sb = persist_w.tile([P, 4, D], BF16, tag="wo")

        # ---- DMA schedule: everything split into <=256KB pieces across
        # queues, ordered so phase B2's inputs (wv + xt first half) land
        # first.
        wvr = wv_d.rearrange("(o p) c -> p o c", p=P)
        wqr = wq_d.rearrange("(o p) c -> p o c", p=P)
        wkr = wk_d.rearrange("(o p) c -> p o c", p=P)
        wor = wo_d.rearrange("(o p) d -> p o d", p=P)
        for j in range(8):
            nc.sync.dma_start(wv_sb[:, j, :], wvr[:, j, :])

        with tc.tile_pool(name="xt", bufs=1) as xt_pool:
            xt = [xt_pool.tile([P, T], BF16, tag=f"xt{j}", name=f"xt{j}") for j in range(8)]

            for th, (c0, c1) in enumerate(((0, 256), (256, 1024), (1024, 2048))):
                for j in range(8):
                    nc.sync.dma_start(xt[j][:, c0:c1], x_d[j * P:(j + 1) * P, c0:c1])
                if th == 0:
                    for j in range(8):
                        nc.sync.dma_start(wq_sb[:, j, :], wqr[:, j, :])
                elif th == 1:
                    for j in range(8):
                        nc.sync.dma_start(wk_sb[:, j, :], wkr[:, j, :])
            for j in range(4):
                nc.sync.dma_start(wo_sb[:, j, :], wor[:, j, :])

            # ---- Phase B2: V' (ones column at 64 of each head) ---------
            with tc.tile_pool(name="vps", bufs=4, space="PSUM") as v_psum:
                for t in range(NKB):
                    nc.gpsimd.memset(vp[t][:], 1.0)
                    ps = v_psum.tile([P, C], FP32, tag="vps")
                    for j in range(8):
                        nc.tensor.matmul(
                            ps[:],
                            xt[j][:, t * P:(t + 1) * P],
                            wv_sb[:, j, :],
                            start=(j == 0), stop=(j == 7),
                        )
                    dst = vp[t].rearrange("p (h e) -> p h e", e=65)[:, :, 0:64]
                    src2 = ps.rearrange("p (h e) -> p h e", e=64)
                    copy_px(t, dst, src2)

            # ---- Phases B1 + C + D interleaved -------------------------
            with (
                tc.tile_pool(name="stps", bufs=2, space="PSUM") as st_pool,
                tc.tile_pool(name="csA", bufs=2, space="PSUM") as csA_pool,
                tc.tile_pool(name="csB", bufs=2, space="PSUM") as csB_pool,
                tc.tile_pool(name="epool", bufs=6) as e_pool,
                tc.tile_pool(name="npool", bufs=2) as n_pool,
                tc.tile_pool(name="rdram", bufs=2, space="DRAM") as rdram_pool,
                tc.tile_pool(name="osb", bufs=4) as o_pool,
            ):
                def b1(hp):
                    # Q^T / K^T projections for head pair hp; PSUM borrowed
                    # from the cs pools.  After hp=0 the ScalarE is busy
                    # with exps, so copies go to VectorE only.
                    ci = 0
                    for dst, wsb in ((qt, wq_sb), (kt, wk_sb)):
                        for s in range(NSPAN):
                            pool = csA_pool if ci % 2 == 0 else csB_pool
                            tag = "csA" if ci % 2 == 0 else "csB"
                            ps = pool.tile([P, SPAN], FP32, tag=tag, name="qkps")
                            for j in range(8):
                                nc.tensor.matmul(
                                    ps[:],
                                    wsb[:, j, hp * P:(hp + 1) * P],
                                    xt[j][:, s * SPAN:(s + 1) * SPAN],
                                    start=(j == 0), stop=(j == 7),
                                )
                            dsl = dst[hp][:, s * SPAN:(s + 1) * SPAN]
                            if hp == 0:
                                copy_px(ci, dsl, ps[:])
                            else:
                                nc.vector.tensor_copy(dsl, ps[:])
                            ci += 1

                def att_span(s, hp):
                    hA, hB = 2 * hp, 2 * hp + 1
                    csA = csA_pool.tile([P, SPAN], FP32, tag="csA", name="csA")
                    csB = csB_pool.tile([P, SPAN], FP32, tag="csB", name="csB")
                    nkb = 4 * s + 4
                    pend = None  # (e, w, kb) awaiting ctx matmuls

                    def ctx_mm(pe, pw, pkb):
                        co = 128 * max(0, pkb - 4 * s)
                        nc.tensor.matmul(csA[0:65, co:SPAN],
                                         vp[pkb][:, hA * 65:(hA + 1) * 65],
                                         pe[:, 0:pw],
                                         start=(pkb == 0), stop=(pkb == nkb - 1))
                        nc.tensor.matmul(csB[0:65, co:SPAN],
                                         vp[pkb][:, hB * 65:(hB + 1) * 65],
                                         pe[:, 512:512 + pw],
                                         start=(pkb == 0), stop=(pkb == nkb - 1))

                    for kb in range(nkb):
                        ksl = slice(kb * P, (kb + 1) * P)
                        d = max(0, kb - 4 * s)      # diagonal offset 0..3
                        q0 = s * SPAN + 128 * d     # valid q start
                        w = SPAN - 128 * d          # valid width
                        qsl = slice(q0, (s + 1) * SPAN)
                        st = st_pool.tile([P, 1024], FP32, tag="st")
                        nc.tensor.matmul(st[:, 0:w], kt[hp][0:64, ksl],
                                         qt[hp][0:64, qsl],
                                         start=True, stop=True,
                                         tile_position=(0, 0))
                        nc.tensor.matmul(st[:, 512:512 + w], kt[hp][64:128, ksl],
                                         qt[hp][64:128, qsl],
                                         start=True, stop=True,
                                         tile_position=(64, 0))
                        e = e_pool.tile([P, 1024], BF16, tag="e")
                        if w == SPAN:
                            # contiguous full-width AP (cheaper than 3D view)
                            nc.scalar.activation(e[:, :], st[:, :], Exp, scale=0.125)
                        else:
                            st3 = st.rearrange("p (b q) -> p b q", b=2)[:, :, 0:w]
                            e3 = e.rearrange("p (b q) -> p b q", b=2)[:, :, 0:w]
                            nc.scalar.activation(e3, st3, Exp, scale=0.125)
                        if d > 0 or kb == 4 * s:
                            e3 = e.rearrange("p (b q) -> p b q", b=2)[:, :, 0:w]
                            m3 = mask_sb[:, None, 384:384 + w]
                            nc.vector.tensor_mul(
                                e3, e3, m3.to_broadcast((P, 2, w)))
                        if pend is not None:
                            ctx_mm(*pend)
                        pend = (e, w, kb)
                    ctx_mm(*pend)
                    # normalize: rows 0..63 / row 64 (ones-column rowsum).
                    # reciprocal_approx_fast is broken at nonzero base
                    # partition: broadcast first (DRAM bounce), recip at 0.
                    qsl = slice(s * SPAN, (s + 1) * SPAN)
                    rs = n_pool.tile([P, 1024], FP32, tag="rs")
                    rsA = n_pool.tile([P, SPAN], FP32, tag="rsA")
                    rsB = n_pool.tile([P, SPAN], FP32, tag="rsB")
                    rrA = n_pool.tile([P, SPAN], FP32, tag="rrA")
                    rrB = n_pool.tile([P, SPAN], FP32, tag="rrB")
                    tmpB = n_pool.tile([P, SPAN], BF16, tag="tmpB")
                    nc.vector.tensor_copy(rs[64:65, 0:512], csA[64:65, :])
                    nc.vector.tensor_copy(rs[64:65, 512:1024], csB[64:65, :])
                    rd = rdram_pool.tile([1024], FP32, tag="rd")
                    nc.sync.dma_start(rd[None, :], rs[64:65, :])
                    nc.sync.dma_start(
                        rsA[0:64, :], rd[None, 0:512].to_broadcast((64, 512)))
                    nc.sync.dma_start(
                        rsB[0:64, :], rd[None, 512:1024].to_broadcast((64, 512)))
                    nc.vector.reciprocal_approx_fast(rrA[0:64, :], rsA[0:64, :])
                    nc.vector.reciprocal_approx_fast(rrB[0:64, :], rsB[0:64, :])
                    nc.vector.tensor_mul(ctxT[hp][0:64, qsl],
                                         csA[0:64, :], rrA[0:64, :])
                    nc.vector.tensor_mul(tmpB[0:64, :],
                                         csB[0:64, :], rrB[0:64, :])
                    nc.sync.dma_start(ctxT[hp][64:128, qsl], tmpB[0:64, :])

                def out_store(qb, nh, pss):
                    ot = o_pool.tile([P, SPAN], FP32, tag="osb", name="osb")
                    nc.vector.tensor_copy(ot[:], pss[:])
                    for h2 in range(4):
                        csl = slice(nh * SPAN + h2 * 128, nh * SPAN + (h2 + 1) * 128)
                        nc.sync.dma_start(
                            out_d[qb * P:(qb + 1) * P, csl],
                            ot[:, h2 * 128:(h2 + 1) * 128])

                def out_span(s):
                    # output projection for q-span s; PSUM borrowed from the
                    # st pool (one [128,1024] tile = both D halves)
                    for qq in range(4):
                        qb = 4 * s + qq
                        ops = st_pool.tile([P, 1024], FP32, tag="st", name="ops")
                        pss = [ops[:, 0:SPAN], ops[:, SPAN:2 * SPAN]]
                        for hp in range(4):
                            for nh in range(2):
                                nc.tensor.matmul(
                                    pss[nh][:],
                                    ctxT[hp][:, qb * P:(qb + 1) * P],
                                    wo_sb[:, hp, nh * SPAN:(nh + 1) * SPAN],
                                    start=(hp == 0), stop=(hp == 3),
                                )
                        for nh in range(2):
                            out_store(qb, nh, pss[nh])

                for hp in range(4):
                    b1(hp)
                    for s in range(NSPAN):
                        att_span(s, hp)
                        if hp == 3 and s >= 1:
                            out_span(s - 1)  # delayed: D(s-1) hides norm(s)
                out_span(NSPAN - 1)

    nc.compile()
    return nc


def _get_program():
    global _program
    if _program is None:
        _program = _build()
    return _program


def _make_mask():
    import ml_dtypes
    j = np.arange(1024)[None, :]
    k = np.arange(P)[:, None]
    return np.where(j >= k + 384, 1.0, 0.0).astype(ml_dtypes.bfloat16)


def _make_in_maps(x, Wq, Wk, Wv, Wo):
    import ml_dtypes
    bf16 = ml_dtypes.bfloat16
    mask = _make_mask()
    xt = [np.ascontiguousarray(np.asarray(x[b], np.float32).astype(bf16).T)
          for b in range(x.shape[0])]
    in_maps = []
    for c in range(N_CORES):
        b, g = c // 2, c % 2
        cols = slice(g * C, (g + 1) * C)
        in_maps.append({
            "x": xt[b],
            "wq": np.ascontiguousarray(np.asarray(Wq[:, cols], np.float32).astype(bf16)),
            "wk": np.ascontiguousarray(np.asarray(Wk[:, cols], np.float32).astype(bf16)),
            "wv": np.ascontiguousarray(np.asarray(Wv[:, cols], np.float32).astype(bf16)),
            "wo": np.ascontiguousarray(np.asarray(Wo[cols, :], np.float32).astype(bf16)),
            "mask": mask,
        })
    return in_maps


def _combine(results, bo, B):
    out = np.empty((B, T, D), dtype=np.float32)
    bo = np.asarray(bo, dtype=np.float32)
    for b in range(B):
        out[b] = results[2 * b]["out"] + results[2 * b + 1]["out"] + bo
    return out


def kernel(x, Wq, Wk, Wv, Wo, bo):
    x = np.asarray(x)
    nc = _get_program()
    in_maps = _make_in_maps(x, Wq, Wk, Wv, Wo)
    res = run_bass_kernel_spmd(nc, in_maps, core_ids=list(range(N_CORES)))
    return _combine(res.results, bo, x.shape[0])


def kernel_traced(x, Wq, Wk, Wv, Wo, bo):
    """Like kernel() but also returns the BassKernelResults (with
    exec_time_ns when NTFF tracing is available)."""
    x = np.asarray(x)
    nc = _get_program()
    in_maps = _make_in_maps(x, Wq, Wk, Wv, Wo)
    res = run_bass_kernel_spmd(nc, in_maps, core_ids=list(range(N_CORES)),
                               trace=True)
    return _combine(res.results, bo, x.shape[0]), res


# revision 14
# speedup vs baseline: 1.0394x; 1.0394x over previous
"""Multi-head causal self-attention on 8 Trainium2 NeuronCores.

Problem: B=4, T=2048, D=1024, H=16 heads, Hd=64. fp32.
Sharding: core c handles batch b = c//2 and head-group g = c%2 (8 heads,
512 channels). Each core computes a partial output (its head-group's
contribution to x @ Wo); the host sums head-group pairs and adds bo.

Per-core algorithm (x is pre-transposed on the host, so no on-chip
transposes are needed anywhere):
  x^T  [D=1024, T]   host-side transpose, plain DMA in (split across queues)
  Q^T  [C=512, T]    = matmul(lhsT=Wq chunk, rhs=x^T)   (head h at partitions
  K^T  [C=512, T]      64*(h%2) .. of chunk h//2)
  V'   [T, 8*65]     = matmul(lhsT=x^T chunk, rhs=Wv), per head [V(64) | 1]
  S^T  [k,q]         = matmul(lhsT=K^T block, rhs=Q^T span)  (k on partitions)
  E = exp((S^T+mask)/8)  on ScalarE, PSUM->SBUF
  ctx' [65, q]       = matmul(lhsT=V' block, rhs=E)  accumulated over k blocks
                       row 64 = softmax denominator (ones-column trick)
  ctx^T normalized via reciprocal + DRAM-bounce broadcast + DVE mult
  out  [T, D]        = matmul(lhsT=ctx^T chunk, rhs=Wo chunk), DMA out
Causality: only k-blocks with k0 <= q_span_end are computed; the <=4
diagonal blocks per span get an additive staircase mask.

Schedule: the attention inner loop is ScalarE-bound (exp), so the Q/K
projections are interleaved per head-pair into the attention sweep
(B1(hp) right before att(*, hp)) and the out-projection of span s is
emitted inside the hp=3 sweep — both give the PE dense work while
ScalarE digests exps.  All PSUM flows through three pools (st 2x2
banks, csA/csB 2x1 banks each = 8 banks); B1 and the out-projection
borrow the cs tiles.  ctx matmuls trail the S matmuls by one k-block
so exp has extra slack.
"""

import sys

for _p in ("/opt/trn_rl_repo", "/root/.axon_site/_ro/trn_rl_repo"):
    if _p not in sys.path:
        sys.path.append(_p)

import numpy as np

import concourse.bacc as bacc
import concourse.mybir as mybir
import concourse.tile as tile
from concourse.bass_utils import run_bass_kernel_spmd

FP32 = mybir.dt.float32
BF16 = mybir.dt.bfloat16
P = 128
T = 2048  # sequence length
D = 1024  # model dim
C = 512   # channels per core (8 heads)
H = 8     # heads per core
HD = 64   # head dim
N_CORES = 8
NSPAN = 4          # q spans of 512
SPAN = 512
NKB = 16           # k blocks of 128

_program = None


def _build():
    nc = bacc.Bacc()
    x_d = nc.declare_dram_parameter("x", [D, T], BF16, isOutput=False)  # x^T
    wq_d = nc.declare_dram_parameter("wq", [D, C], BF16, isOutput=False)
    wk_d = nc.declare_dram_parameter("wk", [D, C], BF16, isOutput=False)
    wv_d = nc.declare_dram_parameter("wv", [D, C], BF16, isOutput=False)
    wo_d = nc.declare_dram_parameter("wo", [C, D], BF16, isOutput=False)
    mask_d = nc.declare_dram_parameter("mask", [P, 1024], BF16, isOutput=False)
    out_d = nc.declare_dram_parameter("out", [T, D], FP32, isOutput=True)

    Exp = mybir.ActivationFunctionType.Exp

    def copy_px(idx, dst, src):
        # alternate PSUM->SBUF copies between ScalarE and VectorE
        if idx % 2 == 0:
            nc.scalar.copy(dst, src)
        else:
            nc.vector.tensor_copy(dst, src)

    from contextlib import ExitStack

    with tile.TileContext(nc) as tc, ExitStack() as persist:
        const_pool = persist.enter_context(tc.tile_pool(name="const", bufs=1))
        qkt_pool = persist.enter_context(tc.tile_pool(name="qkt", bufs=1))
        vp_pool = persist.enter_context(tc.tile_pool(name="vp", bufs=1))
        persist_w = persist.enter_context(tc.tile_pool(name="pw", bufs=1))
        ctxT_pool = persist.enter_context(tc.tile_pool(name="ctxT", bufs=1))

        mask_sb = const_pool.tile([P, 1024], BF16, tag="mask")
        nc.sync.dma_start(mask_sb[:], mask_d[:])
        qt = [qkt_pool.tile([P, T], BF16, tag=f"qt{i}", name=f"qt{i}") for i in range(4)]
        kt = [qkt_pool.tile([P, T], BF16, tag=f"kt{i}", name=f"kt{i}") for i in range(4)]
        vp = [vp_pool.tile([P, H * 65], BF16, tag=f"vp{t}", name=f"vp{t}") for t in range(NKB)]
        ctxT = [ctxT_pool.tile([P, T], BF16, tag=f"ct{i}", name=f"ct{i}")
                for i in range(4)]
        wq_sb = persist_w.tile([P, 8, C], BF16, tag="wq")
        wk_sb = persist_w.tile([P, 8, C], BF16, tag="wk")
        wv_sb = persist_w.tile([P, 8, C], BF16, tag="wv")
        wo_sb = persist_w.tile([P, 4, D], BF16, tag="wo")

        # ---- DMA schedule: everything split into <=256KB pieces across
        # queues, ordered so phase B2's inputs (wv + xt first half) land
        # first.
        wvr = wv_d.rearrange("(o p) c -> p o c", p=P)
        wqr = wq_d.rearrange("(o p) c -> p o c", p=P)
        wkr = wk_d.rearrange("(o p) c -> p o c", p=P)
        wor = wo_d.rearrange("(o p) d -> p o d", p=P)
        for j in range(8):
            nc.sync.dma_start(wv_sb[:, j, :], wvr[:, j, :])

        with tc.tile_pool(name="xt", bufs=1) as xt_pool:
            xt = [xt_pool.tile([P, T], BF16, tag=f"xt{j}", name=f"xt{j}") for j in range(8)]

            for c0, c1 in ((0, 256), (256, 1024), (1024, 2048)):
                for j in range(8):
                    nc.sync.dma_start(xt[j][:, c0:c1], x_d[j * P:(j + 1) * P, c0:c1])
            for j in range(8):
                nc.sync.dma_start(wq_sb[:, j, :], wqr[:, j, :])
            for j in range(8):
                nc.sync.dma_start(wk_sb[:, j, :], wkr[:, j, :])
            for j in range(4):
                nc.sync.dma_start(wo_sb[:, j, :], wor[:, j, :])

            # ---- Phase B2: V' (ones column at 64 of each head) ---------
            with tc.tile_pool(name="vps", bufs=4, space="PSUM") as v_psum:
                for t in range(NKB):
                    nc.gpsimd.memset(vp[t][:], 1.0)
                    ps = v_psum.tile([P, C], FP32, tag="vps")
                    for j in range(8):
                        nc.tensor.matmul(
                            ps[:],
                            xt[j][:, t * P:(t + 1) * P],
                            wv_sb[:, j, :],
                            start=(j == 0), stop=(j == 7),
                        )
                    dst = vp[t].rearrange("p (h e) -> p h e", e=65)[:, :, 0:64]
                    src2 = ps.rearrange("p (h e) -> p h e", e=64)
                    copy_px(t, dst, src2)

            # ---- Phases B1 + C + D interleaved -------------------------
            with (
                tc.tile_pool(name="stps", bufs=2, space="PSUM") as st_pool,
                tc.tile_pool(name="csA", bufs=2, space="PSUM") as csA_pool,
                tc.tile_pool(name="csB", bufs=2, space="PSUM") as csB_pool,
                tc.tile_pool(name="epool", bufs=6) as e_pool,
                tc.tile_pool(name="npool", bufs=2) as n_pool,
                tc.tile_pool(name="rdram", bufs=2, space="DRAM") as rdram_pool,
                tc.tile_pool(name="osb", bufs=4) as o_pool,
            ):
                def b1_group(hp, gi):
                    # one Q^T or K^T projection group (head pair hp, span
                    # gi%4, Q if gi<4 else K); PSUM borrowed from the cs
                    # pools.  After hp=0 the ScalarE is busy with exps, so
                    # copies go to VectorE only.
                    dst, wsb = ((qt, wq_sb), (kt, wk_sb))[gi // 4]
                    s = gi % 4
                    pool = csA_pool if gi % 2 == 0 else csB_pool
                    tag = "csA" if gi % 2 == 0 else "csB"
                    ps = pool.tile([P, SPAN], FP32, tag=tag, name="qkps")
                    for j in range(8):
                        nc.tensor.matmul(
                            ps[:],
                            wsb[:, j, hp * P:(hp + 1) * P],
                            xt[j][:, s * SPAN:(s + 1) * SPAN],
                            start=(j == 0), stop=(j == 7),
                        )
                    dsl = dst[hp][:, s * SPAN:(s + 1) * SPAN]
                    if hp == 0:
                        copy_px(gi, dsl, ps[:])
                    else:
                        nc.vector.tensor_copy(dsl, ps[:])

                def att_span(s, hp):
                    hA, hB = 2 * hp, 2 * hp + 1
                    csA = csA_pool.tile([P, SPAN], FP32, tag="csA", name="csA")
                    csB = csB_pool.tile([P, SPAN], FP32, tag="csB", name="csB")
                    nkb = 4 * s + 4
                    pend = None  # (e, w, kb) awaiting ctx matmuls

                    def ctx_mm(pe, pw, pkb):
                        co = 128 * max(0, pkb - 4 * s)
                        nc.tensor.matmul(csA[0:65, co:SPAN],
                                         vp[pkb][:, hA * 65:(hA + 1) * 65],
                                         pe[:, 0:pw],
                                         start=(pkb == 0), stop=(pkb == nkb - 1))
                        nc.tensor.matmul(csB[0:65, co:SPAN],
                                         vp[pkb][:, hB * 65:(hB + 1) * 65],
                                         pe[:, 512:512 + pw],
                                         start=(pkb == 0), stop=(pkb == nkb - 1))

                    for kb in range(nkb):
                        ksl = slice(kb * P, (kb + 1) * P)
                        d = max(0, kb - 4 * s)      # diagonal offset 0..3
                        q0 = s * SPAN + 128 * d     # valid q start
                        w = SPAN - 128 * d          # valid width
                        qsl = slice(q0, (s + 1) * SPAN)
                        st = st_pool.tile([P, 1024], FP32, tag="st")
                        nc.tensor.matmul(st[:, 0:w], kt[hp][0:64, ksl],
                                         qt[hp][0:64, qsl],
                                         start=True, stop=True,
                                         tile_position=(0, 0))
                        nc.tensor.matmul(st[:, 512:512 + w], kt[hp][64:128, ksl],
                                         qt[hp][64:128, qsl],
                                         start=True, stop=True,
                                         tile_position=(64, 0))
                        e = e_pool.tile([P, 1024], BF16, tag="e")
                        if w == SPAN:
                            # contiguous full-width AP (cheaper than 3D view)
                            nc.scalar.activation(e[:, :], st[:, :], Exp, scale=0.125)
                        else:
                            st3 = st.rearrange("p (b q) -> p b q", b=2)[:, :, 0:w]
                            e3 = e.rearrange("p (b q) -> p b q", b=2)[:, :, 0:w]
                            nc.scalar.activation(e3, st3, Exp, scale=0.125)
                        if d > 0 or kb == 4 * s:
                            e3 = e.rearrange("p (b q) -> p b q", b=2)[:, :, 0:w]
                            m3 = mask_sb[:, None, 384:384 + w]
                            nc.vector.tensor_mul(
                                e3, e3, m3.to_broadcast((P, 2, w)))
                        if pend is not None:
                            ctx_mm(*pend)
                        pend = (e, w, kb)
                    ctx_mm(*pend)
                    # normalize: rows 0..63 / row 64 (ones-column rowsum).
                    # reciprocal_approx_fast is broken at nonzero base
                    # partition: broadcast first (DRAM bounce), recip at 0.
                    qsl = slice(s * SPAN, (s + 1) * SPAN)
                    rs = n_pool.tile([P, 1024], FP32, tag="rs")
                    rsA = n_pool.tile([P, SPAN], FP32, tag="rsA")
                    rsB = n_pool.tile([P, SPAN], FP32, tag="rsB")
                    rrA = n_pool.tile([P, SPAN], FP32, tag="rrA")
                    rrB = n_pool.tile([P, SPAN], FP32, tag="rrB")
                    tmpB = n_pool.tile([P, SPAN], BF16, tag="tmpB")
                    nc.vector.tensor_copy(rs[64:65, 0:512], csA[64:65, :])
                    nc.vector.tensor_copy(rs[64:65, 512:1024], csB[64:65, :])
                    rd = rdram_pool.tile([1024], FP32, tag="rd")
                    nc.sync.dma_start(rd[None, :], rs[64:65, :])
                    nc.sync.dma_start(
                        rsA[0:64, :], rd[None, 0:512].to_broadcast((64, 512)))
                    nc.sync.dma_start(
                        rsB[0:64, :], rd[None, 512:1024].to_broadcast((64, 512)))
                    nc.vector.reciprocal_approx_fast(rrA[0:64, :], rsA[0:64, :])
                    nc.vector.reciprocal_approx_fast(rrB[0:64, :], rsB[0:64, :])
                    nc.vector.tensor_mul(ctxT[hp][0:64, qsl],
                                         csA[0:64, :], rrA[0:64, :])
                    nc.vector.tensor_mul(tmpB[0:64, :],
                                         csB[0:64, :], rrB[0:64, :])
                    nc.sync.dma_start(ctxT[hp][64:128, qsl], tmpB[0:64, :])

                def out_store(qb, nh, pss):
                    ot = o_pool.tile([P, SPAN], FP32, tag="osb", name="osb")
                    nc.vector.tensor_copy(ot[:], pss[:])
                    for h2 in range(4):
                        csl = slice(nh * SPAN + h2 * 128, nh * SPAN + (h2 + 1) * 128)
                        nc.sync.dma_start(
                            out_d[qb * P:(qb + 1) * P, csl],
                            ot[:, h2 * 128:(h2 + 1) * 128])

                def out_span(s, use_st=False):
                    # output projection for q-span s.  PSUM borrowed from
                    # the cs pools normally; the final span borrows the st
                    # pool instead (cs banks are still pinned by the last
                    # normalization there).
                    for qq in range(4):
                        qb = 4 * s + qq
                        if use_st:
                            ops = st_pool.tile([P, 1024], FP32, tag="st", name="ops")
                            pss = [ops[:, 0:SPAN], ops[:, SPAN:2 * SPAN]]
                        else:
                            pss = [csA_pool.tile([P, SPAN], FP32, tag="csA", name="opsA"),
                                   csB_pool.tile([P, SPAN], FP32, tag="csB", name="opsB")]
                        for hp in range(4):
                            for nh in range(2):
                                nc.tensor.matmul(
                                    pss[nh][:],
                                    ctxT[hp][:, qb * P:(qb + 1) * P],
                                    wo_sb[:, hp, nh * SPAN:(nh + 1) * SPAN],
                                    start=(hp == 0), stop=(hp == 3),
                                )
                        for nh in range(2):
                            out_store(qb, nh, pss[nh])

                for gi in range(8):
                    b1_group(0, gi)
                # b1(hp+1) groups woven between att spans of sweep hp so
                # the PE has dense work while ScalarE digests exps
                inserts = {0: (0, 1, 2), 1: (3, 4, 5), 2: (6, 7)}
                for hp in range(4):
                    for s in range(NSPAN):
                        att_span(s, hp)
                        if hp < 3:
                            for gi in inserts.get(s, ()):
                                b1_group(hp + 1, gi)
                        elif s >= 1:
                            out_span(s - 1)  # delayed: D(s-1) hides norm(s)
                out_span(NSPAN - 1, use_st=True)

    nc.compile()
    return nc


def _get_program():
    global _program
    if _program is None:
        _program = _build()
    return _program


def _make_mask():
    import ml_dtypes
    j = np.arange(1024)[None, :]
    k = np.arange(P)[:, None]
    return np.where(j >= k + 384, 1.0, 0.0).astype(ml_dtypes.bfloat16)


def _make_in_maps(x, Wq, Wk, Wv, Wo):
    import ml_dtypes
    bf16 = ml_dtypes.bfloat16
    mask = _make_mask()
    xt = [np.ascontiguousarray(np.asarray(x[b], np.float32).astype(bf16).T)
          for b in range(x.shape[0])]
    in_maps = []
    for c in range(N_CORES):
        b, g = c // 2, c % 2
        cols = slice(g * C, (g + 1) * C)
        in_maps.append({
            "x": xt[b],
            "wq": np.ascontiguousarray(np.asarray(Wq[:, cols], np.float32).astype(bf16)),
            "wk": np.ascontiguousarray(np.asarray(Wk[:, cols], np.float32).astype(bf16)),
            "wv": np.ascontiguousarray(np.asarray(Wv[:, cols], np.float32).astype(bf16)),
            "wo": np.ascontiguousarray(np.asarray(Wo[cols, :], np.float32).astype(bf16)),
            "mask": mask,
        })
    return in_maps


def _combine(results, bo, B):
    out = np.empty((B, T, D), dtype=np.float32)
    bo = np.asarray(bo, dtype=np.float32)
    for b in range(B):
        out[b] = results[2 * b]["out"] + results[2 * b + 1]["out"] + bo
    return out


def kernel(x, Wq, Wk, Wv, Wo, bo):
    x = np.asarray(x)
    nc = _get_program()
    in_maps = _make_in_maps(x, Wq, Wk, Wv, Wo)
    res = run_bass_kernel_spmd(nc, in_maps, core_ids=list(range(N_CORES)))
    return _combine(res.results, bo, x.shape[0])


def kernel_traced(x, Wq, Wk, Wv, Wo, bo):
    """Like kernel() but also returns the BassKernelResults (with
    exec_time_ns when NTFF tracing is available)."""
    x = np.asarray(x)
    nc = _get_program()
    in_maps = _make_in_maps(x, Wq, Wk, Wv, Wo)
    res = run_bass_kernel_spmd(nc, in_maps, core_ids=list(range(N_CORES)),
                               trace=True)
    return _combine(res.results, bo, x.shape[0]), res


# revision 20
# speedup vs baseline: 1.0611x; 1.0209x over previous
"""Multi-head causal self-attention on 8 Trainium2 NeuronCores.

Problem: B=4, T=2048, D=1024, H=16 heads, Hd=64. fp32.
Sharding: core c handles batch b = c//2 and head-group g = c%2 (8 heads,
512 channels). Each core computes a partial output (its head-group's
contribution to x @ Wo); the host sums head-group pairs and adds bo.

Per-core algorithm (x is pre-transposed on the host, so no on-chip
transposes are needed anywhere):
  x^T  [D=1024, T]   host-side transpose, plain DMA in (split across queues)
  Q^T  [C=512, T]    = matmul(lhsT=Wq chunk, rhs=x^T)   (head h at partitions
  K^T  [C=512, T]      64*(h%2) .. of chunk h//2)
  V'   [T, 8*65]     = matmul(lhsT=x^T chunk, rhs=Wv), per head [V(64) | 1]
  S^T  [k,q]         = matmul(lhsT=K^T block, rhs=Q^T span)  (k on partitions)
  E = exp((S^T+mask)/8)  on ScalarE, PSUM->SBUF
  ctx' [65, q]       = matmul(lhsT=V' block, rhs=E)  accumulated over k blocks
                       row 64 = softmax denominator (ones-column trick)
  ctx^T normalized via reciprocal + DRAM-bounce broadcast + DVE mult
  out  [T, D]        = matmul(lhsT=ctx^T chunk, rhs=Wo chunk), DMA out
Causality: only k-blocks with k0 <= q_span_end are computed; the <=4
diagonal blocks per span get an additive staircase mask.

Schedule: the attention inner loop is ScalarE-bound (exp), so the Q/K
projections are interleaved per head-pair into the attention sweep
(B1(hp) right before att(*, hp)) and the out-projection of span s is
emitted inside the hp=3 sweep — both give the PE dense work while
ScalarE digests exps.  All PSUM flows through three pools (st 2x2
banks, csA/csB 2x1 banks each = 8 banks); B1 and the out-projection
borrow the cs tiles.  ctx matmuls trail the S matmuls by one k-block
so exp has extra slack.
"""

import sys

for _p in ("/opt/trn_rl_repo", "/root/.axon_site/_ro/trn_rl_repo"):
    if _p not in sys.path:
        sys.path.append(_p)

import numpy as np

import concourse.bacc as bacc
import concourse.mybir as mybir
import concourse.tile as tile
from concourse.bass_utils import run_bass_kernel_spmd

FP32 = mybir.dt.float32
BF16 = mybir.dt.bfloat16
P = 128
T = 2048  # sequence length
D = 1024  # model dim
C = 512   # channels per core (8 heads)
H = 8     # heads per core
HD = 64   # head dim
N_CORES = 8
NSPAN = 4          # q spans of 512
SPAN = 512
NKB = 16           # k blocks of 128

_program = None


def _build():
    nc = bacc.Bacc()
    x_d = nc.declare_dram_parameter("x", [D, T], BF16, isOutput=False)  # x^T
    wq_d = nc.declare_dram_parameter("wq", [D, C], BF16, isOutput=False)
    wk_d = nc.declare_dram_parameter("wk", [D, C], BF16, isOutput=False)
    wv_d = nc.declare_dram_parameter("wv", [D, C], BF16, isOutput=False)
    wo_d = nc.declare_dram_parameter("wo", [C, D], BF16, isOutput=False)
    mask_d = nc.declare_dram_parameter("mask", [P, 1024], BF16, isOutput=False)
    out_d = nc.declare_dram_parameter("out", [T, D], FP32, isOutput=True)

    Exp = mybir.ActivationFunctionType.Exp

    def copy_px(idx, dst, src):
        # alternate PSUM->SBUF copies between ScalarE and VectorE
        if idx % 2 == 0:
            nc.scalar.copy(dst, src)
        else:
            nc.vector.tensor_copy(dst, src)

    from contextlib import ExitStack

    with tile.TileContext(nc) as tc, ExitStack() as persist:
        const_pool = persist.enter_context(tc.tile_pool(name="const", bufs=1))
        qkt_pool = persist.enter_context(tc.tile_pool(name="qkt", bufs=1))
        vp_pool = persist.enter_context(tc.tile_pool(name="vp", bufs=1))
        persist_w = persist.enter_context(tc.tile_pool(name="pw", bufs=1))
        ctxT_pool = persist.enter_context(tc.tile_pool(name="ctxT", bufs=1))

        mask_sb = const_pool.tile([P, 1024], BF16, tag="mask")
        nc.sync.dma_start(mask_sb[:], mask_d[:])
        qt = [qkt_pool.tile([P, T], BF16, tag=f"qt{i}", name=f"qt{i}") for i in range(4)]
        kt = [qkt_pool.tile([P, T], BF16, tag=f"kt{i}", name=f"kt{i}") for i in range(4)]
        vp = [vp_pool.tile([P, H * 65], BF16, tag=f"vp{t}", name=f"vp{t}") for t in range(NKB)]
        ctxT = [ctxT_pool.tile([P, T], BF16, tag=f"ct{i}", name=f"ct{i}")
                for i in range(4)]
        wq_sb = persist_w.tile([P, 8, C], BF16, tag="wq")
        wk_sb = persist_w.tile([P, 8, C], BF16, tag="wk")
        wv_sb = persist_w.tile([P, 8, C], BF16, tag="wv")
        wo_sb = persist_w.tile([P, 4, D], BF16, tag="wo")

        # ---- DMA schedule: everything split into <=256KB pieces across
        # queues, ordered so phase B2's inputs (wv + xt first half) land
        # first.
        wvr = wv_d.rearrange("(o p) c -> p o c", p=P)
        wqr = wq_d.rearrange("(o p) c -> p o c", p=P)
        wkr = wk_d.rearrange("(o p) c -> p o c", p=P)
        wor = wo_d.rearrange("(o p) d -> p o d", p=P)
        for j in range(8):
            nc.sync.dma_start(wv_sb[:, j, :], wvr[:, j, :])

        with tc.tile_pool(name="xt", bufs=1) as xt_pool:
            xt = [xt_pool.tile([P, T], BF16, tag=f"xt{j}", name=f"xt{j}") for j in range(8)]

            # spread DMA-issue across idle engine queues (each dma_start
            # costs ~600ns of serial issue time on its engine)
            for j in range(8):
                nc.sync.dma_start(xt[j][:, 0:256], x_d[j * P:(j + 1) * P, 0:256])
            for j in range(8):
                nc.scalar.dma_start(xt[j][:, 256:1024], x_d[j * P:(j + 1) * P, 256:1024])
            for j in range(8):
                nc.gpsimd.dma_start(xt[j][:, 1024:2048], x_d[j * P:(j + 1) * P, 1024:2048])
            for j in range(8):
                nc.scalar.dma_start(wq_sb[:, j, :], wqr[:, j, :])
            for j in range(8):
                nc.gpsimd.dma_start(wk_sb[:, j, :], wkr[:, j, :])
            for j in range(4):
                nc.gpsimd.dma_start(wo_sb[:, j, :], wor[:, j, :])

            # ---- Phase B2: V' (ones column at 64 of each head) ---------
            with tc.tile_pool(name="vps", bufs=4, space="PSUM") as v_psum:
                for t in range(NKB):
                    nc.gpsimd.memset(vp[t][:], 1.0)
                    ps = v_psum.tile([P, C], FP32, tag="vps")
                    for j in range(8):
                        nc.tensor.matmul(
                            ps[:],
                            xt[j][:, t * P:(t + 1) * P],
                            wv_sb[:, j, :],
                            start=(j == 0), stop=(j == 7),
                        )
                    dst = vp[t].rearrange("p (h e) -> p h e", e=65)[:, :, 0:64]
                    src2 = ps.rearrange("p (h e) -> p h e", e=64)
                    copy_px(t, dst, src2)

            # ---- Phases B1 + C + D interleaved -------------------------
            with (
                tc.tile_pool(name="stps", bufs=2, space="PSUM") as st_pool,
                tc.tile_pool(name="csA", bufs=2, space="PSUM") as csA_pool,
                tc.tile_pool(name="csB", bufs=2, space="PSUM") as csB_pool,
                tc.tile_pool(name="epool", bufs=6) as e_pool,
                tc.tile_pool(name="npool", bufs=2) as n_pool,
                tc.tile_pool(name="rdram", bufs=2, space="DRAM") as rdram_pool,
                tc.tile_pool(name="osb", bufs=4) as o_pool,
            ):
                def b1_group(hp, gi):
                    # one Q^T or K^T projection group (head pair hp, span
                    # gi%4, Q if gi<4 else K); PSUM borrowed from the cs
                    # pools.  After hp=0 the ScalarE is busy with exps, so
                    # copies go to VectorE only.
                    dst, wsb = ((qt, wq_sb), (kt, wk_sb))[gi // 4]
                    s = gi % 4
                    pool = csA_pool if gi % 2 == 0 else csB_pool
                    tag = "csA" if gi % 2 == 0 else "csB"
                    ps = pool.tile([P, SPAN], FP32, tag=tag, name="qkps")
                    for j in range(8):
                        nc.tensor.matmul(
                            ps[:],
                            wsb[:, j, hp * P:(hp + 1) * P],
                            xt[j][:, s * SPAN:(s + 1) * SPAN],
                            start=(j == 0), stop=(j == 7),
                        )
                    dsl = dst[hp][:, s * SPAN:(s + 1) * SPAN]
                    if hp == 0:
                        copy_px(gi, dsl, ps[:])
                    else:
                        nc.vector.tensor_copy(dsl, ps[:])

                def att_span(s, hp):
                    hA, hB = 2 * hp, 2 * hp + 1
                    csA = csA_pool.tile([P, SPAN], FP32, tag="csA", name="csA")
                    csB = csB_pool.tile([P, SPAN], FP32, tag="csB", name="csB")
                    nkb = 4 * s + 4
                    pend = None  # (e, w, kb) awaiting ctx matmuls

                    def ctx_mm(pe, pw, pkb):
                        co = 128 * max(0, pkb - 4 * s)
                        nc.tensor.matmul(csA[0:65, co:SPAN],
                                         vp[pkb][:, hA * 65:(hA + 1) * 65],
                                         pe[:, 0:pw],
                                         start=(pkb == 0), stop=(pkb == nkb - 1))
                        nc.tensor.matmul(csB[0:65, co:SPAN],
                                         vp[pkb][:, hB * 65:(hB + 1) * 65],
                                         pe[:, 512:512 + pw],
                                         start=(pkb == 0), stop=(pkb == nkb - 1))

                    for kb in range(nkb):
                        ksl = slice(kb * P, (kb + 1) * P)
                        d = max(0, kb - 4 * s)      # diagonal offset 0..3
                        q0 = s * SPAN + 128 * d     # valid q start
                        w = SPAN - 128 * d          # valid width
                        qsl = slice(q0, (s + 1) * SPAN)
                        st = st_pool.tile([P, 1024], FP32, tag="st")
                        nc.tensor.matmul(st[:, 0:w], kt[hp][0:64, ksl],
                                         qt[hp][0:64, qsl],
                                         start=True, stop=True,
                                         tile_position=(0, 0))
                        nc.tensor.matmul(st[:, 512:512 + w], kt[hp][64:128, ksl],
                                         qt[hp][64:128, qsl],
                                         start=True, stop=True,
                                         tile_position=(64, 0))
                        e = e_pool.tile([P, 1024], BF16, tag="e")
                        if w == SPAN:
                            # contiguous full-width AP (cheaper than 3D view)
                            nc.scalar.activation(e[:, :], st[:, :], Exp, scale=0.125)
                        else:
                            st3 = st.rearrange("p (b q) -> p b q", b=2)[:, :, 0:w]
                            e3 = e.rearrange("p (b q) -> p b q", b=2)[:, :, 0:w]
                            nc.scalar.activation(e3, st3, Exp, scale=0.125)
                        if d > 0 or kb == 4 * s:
                            e3 = e.rearrange("p (b q) -> p b q", b=2)[:, :, 0:w]
                            m3 = mask_sb[:, None, 384:384 + w]
                            nc.vector.tensor_mul(
                                e3, e3, m3.to_broadcast((P, 2, w)))
                        if pend is not None:
                            ctx_mm(*pend)
                        pend = (e, w, kb)
                    ctx_mm(*pend)
                    # normalize: rows 0..63 / row 64 (ones-column rowsum).
                    # reciprocal_approx_fast is broken at nonzero base
                    # partition: broadcast first (DRAM bounce), recip at 0.
                    qsl = slice(s * SPAN, (s + 1) * SPAN)
                    rs = n_pool.tile([P, 1024], FP32, tag="rs")
                    rsAB = n_pool.tile([P, 1024], FP32, tag="rsAB")
                    rrAB = n_pool.tile([P, 1024], FP32, tag="rrAB")
                    tmpB = n_pool.tile([P, SPAN], BF16, tag="tmpB")
                    nc.vector.tensor_copy(rs[64:65, 0:512], csA[64:65, :])
                    nc.vector.tensor_copy(rs[64:65, 512:1024], csB[64:65, :])
                    rd = rdram_pool.tile([1024], FP32, tag="rd")
                    # DRAM-bounce broadcast (SBUF partition-stride-0 DMAs are
                    # rejected); issued on the idle gpsimd queue to keep the
                    # sync queue clear
                    nc.gpsimd.dma_start(rd[None, :], rs[64:65, :])
                    nc.gpsimd.dma_start(
                        rsAB[0:64, :], rd[None, :].to_broadcast((64, 1024)))
                    nc.vector.reciprocal_approx_fast(rrAB[0:64, :], rsAB[0:64, :])
                    nc.vector.tensor_mul(ctxT[hp][0:64, qsl],
                                         csA[0:64, :], rrAB[0:64, 0:512])
                    nc.vector.tensor_mul(tmpB[0:64, :],
                                         csB[0:64, :], rrAB[0:64, 512:1024])
                    nc.gpsimd.dma_start(ctxT[hp][64:128, qsl], tmpB[0:64, :])

                def out_store(qb, pss):
                    # both D halves into one tile -> one contiguous DMA
                    ot = o_pool.tile([P, 2 * SPAN], FP32, tag="osb", name="osb")
                    nc.vector.tensor_copy(ot[:, 0:SPAN], pss[0][:])
                    nc.vector.tensor_copy(ot[:, SPAN:2 * SPAN], pss[1][:])
                    nc.sync.dma_start(out_d[qb * P:(qb + 1) * P, :], ot[:])

                def out_span(s, use_st=False):
                    # output projection for q-span s.  PSUM borrowed from
                    # the cs pools normally; the final span borrows the st
                    # pool instead (cs banks are still pinned by the last
                    # normalization there).
                    for qq in range(4):
                        qb = 4 * s + qq
                        if use_st:
                            ops = st_pool.tile([P, 1024], FP32, tag="st", name="ops")
                            pss = [ops[:, 0:SPAN], ops[:, SPAN:2 * SPAN]]
                        else:
                            pss = [csA_pool.tile([P, SPAN], FP32, tag="csA", name="opsA"),
                                   csB_pool.tile([P, SPAN], FP32, tag="csB", name="opsB")]
                        for hp in range(4):
                            for nh in range(2):
                                nc.tensor.matmul(
                                    pss[nh][:],
                                    ctxT[hp][:, qb * P:(qb + 1) * P],
                                    wo_sb[:, hp, nh * SPAN:(nh + 1) * SPAN],
                                    start=(hp == 0), stop=(hp == 3),
                                )
                        out_store(qb, pss)

                for hp in range(4):
                    for gi in range(8):
                        b1_group(hp, gi)
                    for s in range(NSPAN):
                        att_span(s, hp)
                        if hp == 3 and s >= 1:
                            out_span(s - 1)  # delayed: D(s-1) hides norm(s)
                out_span(NSPAN - 1, use_st=True)

    nc.compile()
    return nc


def _get_program():
    global _program
    if _program is None:
        _program = _build()
    return _program


def _make_mask():
    import ml_dtypes
    j = np.arange(1024)[None, :]
    k = np.arange(P)[:, None]
    return np.where(j >= k + 384, 1.0, 0.0).astype(ml_dtypes.bfloat16)


def _make_in_maps(x, Wq, Wk, Wv, Wo):
    import ml_dtypes
    bf16 = ml_dtypes.bfloat16
    mask = _make_mask()
    xt = [np.ascontiguousarray(np.asarray(x[b], np.float32).astype(bf16).T)
          for b in range(x.shape[0])]
    in_maps = []
    for c in range(N_CORES):
        b, g = c // 2, c % 2
        cols = slice(g * C, (g + 1) * C)
        in_maps.append({
            "x": xt[b],
            "wq": np.ascontiguousarray(np.asarray(Wq[:, cols], np.float32).astype(bf16)),
            "wk": np.ascontiguousarray(np.asarray(Wk[:, cols], np.float32).astype(bf16)),
            "wv": np.ascontiguousarray(np.asarray(Wv[:, cols], np.float32).astype(bf16)),
            "wo": np.ascontiguousarray(np.asarray(Wo[cols, :], np.float32).astype(bf16)),
            "mask": mask,
        })
    return in_maps


def _combine(results, bo, B):
    out = np.empty((B, T, D), dtype=np.float32)
    bo = np.asarray(bo, dtype=np.float32)
    for b in range(B):
        out[b] = results[2 * b]["out"] + results[2 * b + 1]["out"] + bo
    return out


def kernel(x, Wq, Wk, Wv, Wo, bo):
    x = np.asarray(x)
    nc = _get_program()
    in_maps = _make_in_maps(x, Wq, Wk, Wv, Wo)
    res = run_bass_kernel_spmd(nc, in_maps, core_ids=list(range(N_CORES)))
    return _combine(res.results, bo, x.shape[0])


def kernel_traced(x, Wq, Wk, Wv, Wo, bo):
    """Like kernel() but also returns the BassKernelResults (with
    exec_time_ns when NTFF tracing is available)."""
    x = np.asarray(x)
    nc = _get_program()
    in_maps = _make_in_maps(x, Wq, Wk, Wv, Wo)
    res = run_bass_kernel_spmd(nc, in_maps, core_ids=list(range(N_CORES)),
                               trace=True)
    return _combine(res.results, bo, x.shape[0]), res
